# revision 99
# baseline (speedup 1.0000x reference)
"""Trainium2 Bass kernel for nn_BidirectionalAttention (LayerNorm -> QKV -> RoPE ->
attention with 16 persistent-memory KV tokens -> out projection).

Sharding: 8 cores = (batch b=2) x (4 head-pairs). Each core computes its batch's
LayerNorm + QKV for its 2 heads, full attention over n=4096 (+16 pm) keys, and a
partial output projection; the host sums the 4 partials per batch.

v3 (346us, from the 387us v2 baseline):
  - S = K^T Q runs in fp8e4m3 with the DoubleRow perf mode (2x PE throughput).
  - xn is transposed ON-CHIP: 128x128 PE transposes (vs a host identity) into
    bf16 psum (half a bank, rides the QKV "o" ring), drained by DVE 2x-mode
    copies -- no xn DRAM store, no transpose DMAs, no SP-queue serialization.
  - fill window: qc0-h0 runs fully during the QKV phase; qc0-h1 g0-1 are
    pre-emitted and g8-10 retained in SBUF (ppool 17); qc1-h0/h1 g0-3 are
    computed in the fill window, SPILLED to DRAM, and reloaded at phase-2
    start on the then-idle DMA queues. More spills measurably regress: the
    fill waves are saturated and >8 upfront reloads overflow the pgrp ring.
  - per-stream AV matmuls lag S/exp by 2 groups, the next stream's first
    groups are pre-emitted before this stream's AV tail (in-order PE never
    head-of-line blocks the exp pipeline); outproj is deferred one qc.
  - end tail: the last stream runs av_lag=0, its normalize broadcasts
    1/denom via a 1-row PE matmul (ones64^T @ r, SBUF-bounced -- DVE cannot
    read two psum operands), and the last outproj spreads copies/stores
    across ACT/DVE and three DMA queues.
  - known dead ends (all measured): GPSIMD cannot access PSUM (kills cheap
    poly-exp ingest and psum partition_broadcast); DVE TT divide and
    dma_start_transpose-on-ACT-queue fail compile/correctness; fp8 P or V in
    the AV matmul adds ~3-6% error (over the 2e-2 budget); exp offload to
    DVE/Pool via the f16 quartic costs more schedule than it saves.

Self-contained: hardcodes all shapes from the problem spec.
"""
import math
import sys

sys.path.insert(0, "/opt/trn_rl_repo")

import numpy as np
import ml_dtypes

import concourse.bass as bass
import concourse.tile as tile
from concourse import mybir
from concourse.bass_utils import run_bass_kernel_spmd

BF16 = mybir.dt.bfloat16
F16 = mybir.dt.float16
F32 = mybir.dt.float32
FP8 = mybir.dt.float8e4
AF = mybir.ActivationFunctionType
OP = mybir.AluOpType
DR = mybir.MatmulPerfMode.DoubleRow

B, N, D = 2, 4096, 512
H, DH, NPM = 8, 64, 16
BASE, EPS = 10000.0, 1e-5
SCALE = DH ** -0.5
NCORES = 8
M_TOT = N + NPM          # 4112 keys
MCHUNKS = 33             # 32 seq chunks of 128 + 1 pm chunk of 16
NGRP = 11                # exp/AV groups of 3 m-chunks
QC = 8                   # query chunks of 512
NBLK = 32                # n blocks of 128

# exp(z) ~ ((w+QC0)^2+QC1)*((w+QC2)^2+QC3) with w = GAMMA*z, |z| <= 1.45
GAMMA = 0.442920622
QC0_, QC1_, QC2_, QC3_ = 0.168332009, 1.031577065, 0.907563879, 0.116778835
SQ = math.sqrt(GAMMA * SCALE)   # host prescale on each of Q and K

# KT chunk needed by group g (keys [384g, 384(g+1)) vs 512-wide QKV chunks)
KG = [0, 1, 2, 2, 3, 4, 5, 5, 6, 7, 7]
# groups evaluated with the DVE fp16 quartic instead of ACT exp (per stream,
# only for the sequential-phase streams)
POLY_GS = ()


def _split_excess_waits(nc, max_waits=1):
    """walrus in this container rejects >1 sync waits per instruction; hoist
    extras onto same-engine nops inserted just before (same sequencer order)."""
    cnt = 0
    for fn in nc.m.functions:
        for bb in fn.blocks:
            insts = bb.instructions
            i = 0
            while i < len(insts):
                inst = insts[i]
                si = inst.sync_info
                if si is not None and si.on_wait is not None and len(si.on_wait) > max_waits:
                    waits = list(si.on_wait)
                    extra, keep = waits[:-max_waits], waits[-max_waits:]
                    nops = []
                    for j in range(0, len(extra), max_waits):
                        cnt += 1
                        nop = mybir.InstNoOp(name=f"I-waitsplit-{cnt}-{inst.name}",
                                             engine=inst.engine, ins=[], outs=[])
                        nop.sync_info = mybir.SyncInfo(on_wait=extra[j:j + max_waits],
                                                       on_update=[])
                        nc.register_instruction(nop, overwrite=True)
                        nops.append(nop)
                    si.on_wait = keep
                    for k, nop in enumerate(nops):
                        insts.insert(i + k, nop)
                    i += len(nops)
                i += 1
    return cnt


def build(reps=1):
    nc = bass.Bass()

    x_in = nc.dram_tensor("x_in", [N, D], BF16, kind="ExternalInput")
    wq_in = nc.dram_tensor("wq_in", [5 * 128, 384], BF16, kind="ExternalInput")
    wo0_in = nc.dram_tensor("wo0_in", [64, 512], BF16, kind="ExternalInput")
    wo1_in = nc.dram_tensor("wo1_in", [64, 512], BF16, kind="ExternalInput")
    cos_in = nc.dram_tensor("cos_in", [128, N], BF16, kind="ExternalInput")
    sin_in = nc.dram_tensor("sin_in", [128, N], BF16, kind="ExternalInput")
    pmk_in = nc.dram_tensor("pmk_in", [32, 2, 2, NPM], FP8, kind="ExternalInput")
    pmv_in = nc.dram_tensor("pmv_in", [NPM, 130], BF16, kind="ExternalInput")
    msk_in = nc.dram_tensor("msk_in", [128, NBLK], F32, kind="ExternalInput")
    ones_in = nc.dram_tensor("ones_in", [1, N], BF16, kind="ExternalInput")
    id_in = nc.dram_tensor("id_in", [128, 128], BF16, kind="ExternalInput")
    out_p = nc.dram_tensor("out_p", [N, D], F32, kind="ExternalOutput")

    import contextlib
    with tile.TileContext(nc) as tc:
      with (tc.For_i(0, reps, 1) if reps > 1 else contextlib.nullcontext()):
        with (
            tc.tile_pool(name="persist", bufs=1) as pers,
            tc.tile_pool(name="okpool", bufs=6) as okpool,
            tc.tile_pool(name="dram", bufs=1, space="DRAM") as dr,
            tc.tile_pool(name="spool", bufs=2, space="PSUM") as spool,
            tc.tile_pool(name="opool", bufs=2, space="PSUM") as opool,
            tc.tile_pool(name="ppool", bufs=17) as ppool,
            tc.tile_pool(name="rpool", bufs=2) as rpool,
            tc.tile_pool(name="rbpool", bufs=2) as rbpool,
            tc.tile_pool(name="fpool", bufs=3) as fpool,
            tc.tile_pool(name="rdram", bufs=4, space="DRAM") as rdram,
        ):
            w_sb = pers.tile([128, 5, 384], BF16)
            nc.sync.dma_start(out=w_sb, in_=wq_in.rearrange("(kc p) m -> p kc m", p=128))
            wo_sb = pers.tile([128, 512], BF16)
            nc.sync.dma_start(out=wo_sb[0:64, :], in_=wo0_in[:, :])
            nc.sync.dma_start(out=wo_sb[64:128, :], in_=wo1_in[:, :])
            msk_sb = pers.tile([128, NBLK], F32)
            nc.sync.dma_start(out=msk_sb, in_=msk_in[:, :])
            eps_sb = pers.tile([128, 1], F32)
            nc.vector.memset(eps_sb, EPS)
            ident = pers.tile([128, 128], BF16)
            nc.sync.dma_start(out=ident, in_=id_in[:, :])
            ones64 = pers.tile([1, 64], F32)
            nc.vector.memset(ones64, 1.0)
            # fp8 Q^T/K^T in DoubleRow layout [32p, h, j, n]: dim d = p + 32j
            QT8 = pers.tile([32, 2, 2, N], FP8)
            KT8 = pers.tile([32, 2, 2, M_TOT], FP8)
            Vnat = pers.tile([128, MCHUNKS, 130], BF16)  # per m-chunk: [v_g0(64), m, v_g1(64), m]
            nc.sync.dma_start(out=KT8[:, :, :, N:M_TOT], in_=pmk_in[:, :, :, :])
            nc.sync.dma_start(out=Vnat[0:NPM, 32, :], in_=pmv_in[:, :])

            # ---------------- P0-P2: load x, LayerNorm (in place), DRAM roundtrip transpose
            # DMA queue assignment: the serial per-queue cost gates the whole
            # QKV->RoPE->KT chain, so spread x loads / xn stores / transposes
            # across the SP, Pool and ACT DGE queues, and load cos/sin first.
            with tc.tile_pool(name="xnt", bufs=1) as xntp:
              xnT = [xntp.tile([128, N], BF16, tag=f"xnT{kc}", name=f"xnT{kc}")
                     for kc in range(4)]
              cos_sb = xntp.tile([128, N], BF16, tag="cos")
              sin_sb = xntp.tile([128, N], BF16, tag="sin")
              ones_row = xntp.tile([1, N], BF16, tag="ones")
              nc.sync.dma_start(out=ones_row, in_=ones_in[:, :])
              import contextlib as _ctx
              _es = _ctx.ExitStack()
              xp = _es.enter_context(tc.tile_pool(name="xpool", bufs=1))
              lnp = _es.enter_context(tc.tile_pool(name="lnp", bufs=4))
              if True:
                xr = x_in.rearrange("(t p) d -> p t d", p=128)
                xch = []
                for hc in range(8):
                    xt = xp.tile([128, 4, D], BF16, tag=f"x{hc % 4}", name=f"x{hc}")
                    xch.append(xt)
                    nc.scalar.dma_start(out=xt, in_=xr[:, hc * 4:(hc + 1) * 4, :])
                    if hc == 1:
                        nc.sync.dma_start(out=cos_sb, in_=cos_in[:, :])
                        nc.sync.dma_start(out=sin_sb, in_=sin_in[:, :])
                scr = xp.tile([128, D], BF16, tag="scr")  # ACT-stats discard buffer
                inv_d = 1.0 / D

                def emit_ln(hc):
                    xt = xch[hc]
                    for tt in range(4):
                        mv = lnp.tile([128, 2], F32, tag="mv")
                        rstd = lnp.tile([128, 1], F32, tag="rstd")
                        if tt % 2 == 0:  # split LN stats between DVE and ACT
                            stats = lnp.tile([128, 6], F32, tag="stats")
                            nc.vector.bn_stats(out=stats, in_=xt[:, tt, :])
                            nc.vector.bn_aggr(out=mv, in_=stats)
                            nc.scalar.activation(out=rstd, in_=mv[:, 1:2], func=AF.Sqrt,
                                                 bias=eps_sb, scale=1.0)
                        else:
                            sums = lnp.tile([128, 2], F32, tag="sums")
                            nc.scalar.activation(out=scr, in_=xt[:, tt, :], func=AF.Copy,
                                                 accum_out=sums[:, 0:1])
                            nc.scalar.activation(out=scr, in_=xt[:, tt, :], func=AF.Square,
                                                 accum_out=sums[:, 1:2])
                            nc.vector.tensor_scalar(out=mv, in0=sums, scalar1=inv_d,
                                                    scalar2=None, op0=OP.mult, op1=OP.bypass)
                            mu2 = lnp.tile([128, 1], F32, tag="mu2")
                            nc.vector.tensor_tensor(out=mu2, in0=mv[:, 0:1], in1=mv[:, 0:1],
                                                    op=OP.mult)
                            nc.vector.tensor_tensor(out=mv[:, 1:2], in0=mv[:, 1:2], in1=mu2,
                                                    op=OP.subtract)
                            nc.scalar.activation(out=rstd, in_=mv[:, 1:2], func=AF.Sqrt,
                                                 bias=eps_sb, scale=1.0)
                        nc.vector.reciprocal(out=rstd, in_=rstd)
                        nc.vector.tensor_scalar(out=xt[:, tt, :], in0=xt[:, tt, :],
                                                scalar1=mv[:, 0:1], scalar2=rstd,
                                                op0=OP.subtract, op1=OP.mult)
                    # transpose on the PE (bf16 psum out) + ACT copy to SBUF:
                    # no DRAM roundtrip, no SP-queue serialization
                    for kc in range(4):
                        tb = opool.tile([128, 512], BF16, tag="o", name=f"tb{hc}_{kc}")
                        for tt in range(4):
                            nc.tensor.transpose(tb[:, tt * 128:(tt + 1) * 128],
                                                xt[:, tt, kc * 128:(kc + 1) * 128], ident)
                        nc.vector.tensor_copy(out=xnT[kc][:, hc * 512:(hc + 1) * 512], in_=tb)

              # ---------------- attention group emitters
              def emit_group(qc, h, g, use_dve):
                  """S (fp8 DoubleRow) + exp/poly for one (qc, h, g); returns pgrp."""
                  qsl = slice(qc * 512, (qc + 1) * 512)
                  sgrp = spool.tile([128, 1536], F32, tag="s", name=f"s{qc}_{h}_{g}")
                  for j in range(3):
                      mc = 3 * g + j
                      js = slice(j * 512, (j + 1) * 512)
                      if mc < 32:
                          nc.tensor.matmul(sgrp[:, js],
                                           KT8[:, h, :, mc * 128:(mc + 1) * 128],
                                           QT8[:, h, :, qsl],
                                           start=True, stop=True, perf_mode=DR)
                      else:
                          nc.tensor.matmul(sgrp[0:NPM, js],
                                           KT8[:, h, :, N:M_TOT],
                                           QT8[:, h, :, qsl],
                                           start=True, stop=True, perf_mode=DR)
                  pgrp = ppool.tile([128, 1536], BF16, tag="p", name=f"p{qc}_{h}_{g}")
                  if use_dve:
                      # w = GAMMA*SCALE*s folded into the psum ingest
                      t = u = None  # poly path disabled (polp removed)
                      nc.vector.tensor_scalar(out=t, in0=sgrp, scalar1=GAMMA * SCALE,
                                              scalar2=QC0_, op0=OP.mult, op1=OP.add)
                      nc.vector.tensor_scalar(out=u, in0=t, scalar1=QC2_ - QC0_,
                                              scalar2=None, op0=OP.add, op1=OP.bypass)
                      nc.vector.tensor_tensor(out=t, in0=t, in1=t, op=OP.mult)
                      nc.vector.tensor_tensor(out=u, in0=u, in1=u, op=OP.mult)
                      nc.vector.tensor_scalar(out=t, in0=t, scalar1=QC1_,
                                              scalar2=None, op0=OP.add, op1=OP.bypass)
                      nc.vector.tensor_scalar(out=u, in0=u, scalar1=QC3_,
                                              scalar2=None, op0=OP.add, op1=OP.bypass)
                      nc.vector.tensor_tensor(out=pgrp, in0=t, in1=u, op=OP.mult)
                  else:
                      # last group's pm slice has 112 never-written psum rows; exp of
                      # stale-but-finite logits there is never read by AV.
                      nc.scalar.activation(out=pgrp, in_=sgrp, func=AF.Exp,
                                           scale=SCALE)
                  return pgrp

              def emit_av(o_ps, h, g, pgrp, first, last):
                  for j in range(3):
                      mc = 3 * g + j
                      js = slice(j * 512, (j + 1) * 512)
                      if mc < 32:
                          nc.tensor.matmul(o_ps, Vnat[:, mc, 65 * h:65 * h + 65],
                                           pgrp[:, js],
                                           start=(first and j == 0),
                                           stop=(last and j == 2),
                                           skip_group_check=True)
                      else:
                          nc.tensor.matmul(o_ps, Vnat[0:NPM, mc, 65 * h:65 * h + 65],
                                           pgrp[0:NPM, js],
                                           start=False, stop=(last and j == 2),
                                           skip_group_check=True)

              def emit_norm(qc, h, o_ps, opk, last=False):
                  # r = 1/denom ; broadcast via DRAM bounce ; opk_h = numer * r
                  r_sb = rpool.tile([1, 512], F32, tag="r", name=f"r{qc}_{h}")
                  nc.vector.reciprocal(out=r_sb, in_=o_ps[64:65, :])
                  if last:
                      # no more S matmuls follow: broadcast on the idle PE,
                      # skipping the DRAM-bounce latency in the end tail
                      r64p = opool.tile([64, 512], F32, tag="o", name=f"rp{qc}_{h}")
                      nc.tensor.matmul(r64p, ones64, r_sb, start=True, stop=True)
                      r64 = rbpool.tile([64, 512], F32, tag="rb", name=f"rb{qc}_{h}")
                      nc.scalar.copy(out=r64, in_=r64p)
                  else:
                      r_dr = rdram.tile([1, 512], F32, tag="rd", name=f"rd{qc}_{h}")
                      nc.sync.dma_start(out=r_dr[:, :], in_=r_sb)
                      rd_ap = r_dr[:, :]
                      r_bc = bass.AP(tensor=rd_ap.tensor, offset=rd_ap.offset,
                                     ap=[[0, 64]] + list(rd_ap.ap[1:]))
                      r64 = rbpool.tile([64, 512], F32, tag="rb", name=f"rb{qc}_{h}")
                      nc.gpsimd.dma_start(out=r64, in_=r_bc)
                  nc.vector.tensor_tensor(out=opk[h * 64:(h + 1) * 64, :],
                                          in0=o_ps[0:64, :], in1=r64, op=OP.mult)

              def emit_outproj(qc, opk, last=False):
                  for mb in range(4):
                      psf = opool.tile([128, 512], F32, tag="o", name=f"psf{qc}_{mb}")
                      nc.tensor.matmul(psf, opk[:, mb * 128:(mb + 1) * 128],
                                       wo_sb, start=True, stop=True)
                      f_sb = fpool.tile([128, 512], F32, tag="fs", name=f"f{qc}_{mb}")
                      if last and mb % 2 == 1:
                          nc.scalar.copy(out=f_sb, in_=psf)
                      else:
                          nc.vector.tensor_copy(out=f_sb, in_=psf)
                      row0 = qc * 512 + mb * 128
                      if last:
                          eng = (nc.sync, nc.scalar, nc.gpsimd, nc.sync)[mb]
                      else:
                          eng = nc.sync if mb % 2 == 0 else nc.gpsimd
                      eng.dma_start(out=out_p[row0:row0 + 128, :], in_=f_sb)

              # fill-window plan: qc0/h0 all groups, qc0/h1 groups 0..4,
              # batched by the QKV chunk that completes their keys
              fill_plan = {k: [] for k in range(QC)}
              for g in range(NGRP):
                  fill_plan[KG[g]].append((0, 0, g))
              for g in (8, 9, 10):
                  fill_plan[KG[g]].append((0, 1, g))
              # qc1 groups computed in the fill window and spilled to DRAM
              # (SBUF retention is full); reloaded just-in-time in phase 2
              SPILL = [(1, h, g) for h in range(2) for g in range(4)]
              for (sq, sh, sg) in SPILL:
                  fill_plan[max(KG[sg], 1)].append((sq, sh, sg))
              fill_pgrps = {}
              spill_dram = {}

              # ---------------- P3-P5 + fill window: QKV^T GEMM + RoPE + assembly
              # + Vnat, streamed; qc0 attention interleaved per KT-chunk batch
              msk3 = msk_sb[:, :].rearrange("p (c one) -> p c one", one=1)
              nc.vector.tensor_copy(out=Vnat[:, 0:NBLK, 64:65], in_=msk3)
              nc.vector.tensor_copy(out=Vnat[:, 0:NBLK, 129:130], in_=msk3)
              rp = _es.enter_context(tc.tile_pool(name="rope", bufs=1))
              if True:
                for nc8 in range(QC):
                    emit_ln(nc8)
                    sl = slice(nc8 * 512, (nc8 + 1) * 512)
                    ab = []
                    for mi in range(2):
                        psq = opool.tile([128, 512], F32, tag="o", name=f"psq{nc8}_{mi}")
                        for kc in range(4):
                            nc.tensor.matmul(psq, w_sb[:, kc, mi * 128:(mi + 1) * 128],
                                             xnT[kc][:, sl],
                                             start=(kc == 0), stop=False)
                        nc.tensor.matmul(psq, w_sb[0:1, 4, mi * 128:(mi + 1) * 128],
                                         ones_row[0:1, sl], start=False, stop=True)
                        dst = rp.tile([128, 512], BF16, tag=f"ab{mi}", bufs=2,
                                      name=f"ab{mi}_{nc8}")
                        if mi == 0:
                            nc.vector.tensor_copy(out=dst, in_=psq)
                        else:
                            nc.scalar.copy(out=dst, in_=psq)
                        ab.append(dst)
                    A, Bt = ab
                    # RoPE: rotA = A*cos - B*sin ; rotB = B*cos + A*sin
                    t1 = rp.tile([128, 512], BF16, tag="t1", bufs=2)
                    t2 = rp.tile([128, 512], BF16, tag="t2", bufs=2)
                    t3 = rp.tile([128, 512], BF16, tag="t3", bufs=2)
                    t4 = rp.tile([128, 512], BF16, tag="t4", bufs=2)
                    ra = rp.tile([128, 512], BF16, tag="ra", bufs=2)
                    rb = rp.tile([128, 512], BF16, tag="rb", bufs=2)
                    nc.vector.tensor_tensor(out=t1, in0=A, in1=cos_sb[:, sl], op=OP.mult)
                    nc.vector.tensor_tensor(out=t2, in0=Bt, in1=sin_sb[:, sl], op=OP.mult)
                    nc.vector.tensor_tensor(out=t3, in0=Bt, in1=cos_sb[:, sl], op=OP.mult)
                    nc.vector.tensor_tensor(out=t4, in0=A, in1=sin_sb[:, sl], op=OP.mult)
                    nc.vector.tensor_tensor(out=ra, in0=t1, in1=t2, op=OP.subtract)
                    nc.vector.tensor_tensor(out=rb, in0=t3, in1=t4, op=OP.add)
                    # assemble QT8/KT8 fp8 (dim d = p + 32j; j=0 <- ra, j=1 <- rb);
                    # KT gates attention -> DVE; QT is per-qc (slack) -> GPSIMD
                    for h in range(2):
                        nc.gpsimd.tensor_copy(out=QT8[:, h, 0, sl], in_=ra[32 * h:32 * h + 32, :])
                        nc.gpsimd.tensor_copy(out=QT8[:, h, 1, sl], in_=rb[32 * h:32 * h + 32, :])
                        nc.vector.tensor_copy(out=KT8[:, h, 0, sl], in_=ra[64 + 32 * h:96 + 32 * h, :])
                        nc.vector.tensor_copy(out=KT8[:, h, 1, sl], in_=rb[64 + 32 * h:96 + 32 * h, :])
                    # V natural GEMM for this n-range (+ mask fold)
                    for nb in range(nc8 * 4, nc8 * 4 + 4):
                        psv = opool.tile([128, 128], F32, tag="o", name=f"psv{nb}")
                        for kc in range(4):
                            nc.tensor.matmul(psv, xnT[kc][:, nb * 128:(nb + 1) * 128],
                                             w_sb[:, kc, 256:384],
                                             start=(kc == 0), stop=False)
                        nc.tensor.matmul(psv, ones_row[0:1, nb * 128:(nb + 1) * 128],
                                         w_sb[0:1, 4, 256:384], start=False, stop=True)
                        vdst = Vnat[:, nb, :].rearrange("p (g c) -> p g c", c=65)[:, :, 0:64]
                        vsrc = psv.rearrange("p (g c) -> p g c", c=64)
                        nc.vector.tensor_scalar(out=vdst, in0=vsrc,
                                                scalar1=msk_sb[:, nb:nb + 1], scalar2=None,
                                                op0=OP.mult, op1=OP.bypass)
                    # fill window: qc0 S/exp groups whose keys just completed
                    with tc.high_priority():
                        for (fqc, h, g) in fill_plan[nc8]:
                            pg = emit_group(fqc, h, g, use_dve=False)
                            if (fqc, h, g) in SPILL:
                                sd = dr.tile([128, 1536], BF16, tag=f"sp{fqc}{h}{g}",
                                             name=f"spd{fqc}_{h}_{g}")
                                nc.sync.dma_start(out=sd, in_=pg)
                                spill_dram[(fqc, h, g)] = sd
                            else:
                                fill_pgrps[(fqc, h, g)] = pg

              _es.close()

              # ---------------- P6: attention streams (sequential; o_ps ring).
              # Poly-group AVs are deferred to the stream tail so the in-order
              # PE never head-of-line blocks on the DVE poly chain; outproj is
              # deferred by one query chunk for the same reason (opk waits on
              # the DRAM-bounce normalize).
              def emit_stream(qc, h, opk, av_lag=2, premitted=None, tail_hook=None,
                              last=False):
                  # AVs lag the S/exp emission by `av_lag` groups so the
                  # in-order PE never parks AV matmuls (waiting on exp or the
                  # o_ps ring slot) in its 4-deep wait queue ahead of the next
                  # groups' S matmuls; poly-group AVs go last (DVE latency).
                  # `interleave(g)` lets the qc0-h0 retained-AV burst ride
                  # along between this stream's groups instead of clogging PE.
                  o_ps = opool.tile([65, 512], F32, tag="o", name=f"ops{qc}_{h}")
                  pending = []
                  tail = []
                  emitted = [False]

                  def flush_one():
                      g, pgrp = pending.pop(0)
                      emit_av(o_ps, h, g, pgrp, first=not emitted[0], last=False)
                      emitted[0] = True

                  for g in range(NGRP):
                      if premitted is not None and g in premitted:
                          pgrp = premitted[g]
                          use_dve = False
                      else:
                          use_dve = (g in POLY_GS) and not (qc == 0 and h == 0)
                          pgrp = emit_group(qc, h, g, use_dve)
                      if use_dve:
                          tail.append((g, pgrp))
                      else:
                          pending.append((g, pgrp))
                          if len(pending) > av_lag:
                              flush_one()
                  if tail_hook is not None:
                      tail_hook()
                  allrest = pending + tail
                  pending.clear()
                  for i, (g, pgrp) in enumerate(allrest):
                      emit_av(o_ps, h, g, pgrp, first=not emitted[0],
                              last=(i == len(allrest) - 1))
                      emitted[0] = True
                  emit_norm(qc, h, o_ps, opk, last=last)

              with tc.high_priority():
                  opks = {}
                  for qc in range(QC):
                      opks[qc] = okpool.tile([128, 512], BF16, tag="ok", name=f"opk{qc}")
                  # pre-emit two qc0-h1 groups so ACT has work during the
                  # qc0-h0 retained-pgrp AV burst
                  pre01 = {g: emit_group(0, 1, g, use_dve=False) for g in (0, 1)}
                  for g in (8, 9, 10):
                      pre01[g] = fill_pgrps[(0, 1, g)]
                  # reload the spilled qc1 softmax tiles (DMA queues are idle
                  # in phase 2); streams (1,*) start ~16us from here
                  preload = {}
                  for i, (sq, sh, sg) in enumerate(SPILL):
                      pt = ppool.tile([128, 1536], BF16, tag="p",
                                      name=f"pl{sq}_{sh}_{sg}")
                      qeng = nc.scalar if i % 2 == 0 else nc.sync
                      qeng.dma_start(out=pt, in_=spill_dram[(sq, sh, sg)][:, :])
                      preload.setdefault((sq, sh), {})[sg] = pt
                  b_ops = opool.tile([65, 512], F32, tag="o", name="ops0_0")
                  for g in range(NGRP):
                      emit_av(b_ops, 0, g, fill_pgrps[(0, 0, g)], first=(g == 0),
                              last=(g == NGRP - 1))
                  emit_norm(0, 0, b_ops, opks[0])
                  # uniform stream order with 2-group lookahead: the next
                  # stream's first S/exp groups are emitted before this
                  # stream's tail AVs so ACT never idles at the boundary
                  order = [(0, 1)] + [(qc, h) for qc in range(1, QC) for h in range(2)]
                  pres = {(0, 1): pre01}
                  pres.update(preload)

                  def mk_hook(idx):
                      if idx + 1 >= len(order):
                          return None
                      nqc, nh = order[idx + 1]

                      def hook():
                          d = dict(pres.get((nqc, nh), {}))
                          for g in [g for g in range(NGRP) if g not in d][:4]:
                              d[g] = emit_group(nqc, nh, g, use_dve=False)
                          pres[(nqc, nh)] = d
                      return hook

                  for idx, (qc, h) in enumerate(order):
                      emit_stream(qc, h, opks[qc],
                                  av_lag=(0 if idx == len(order) - 1 else 5),
                                  premitted=pres.get((qc, h)),
                                  tail_hook=mk_hook(idx),
                                  last=(idx == len(order) - 1))
                      if h == 0 and qc >= 1:
                          emit_outproj(qc - 1, opks[qc - 1])
                  emit_outproj(QC - 1, opks[QC - 1], last=True)

    _split_excess_waits(nc)
    return nc


_STATE = {}


def _get_nc():
    if "nc" not in _STATE:
        _STATE["nc"] = build()
    return _STATE["nc"]


def _rope_tables():
    inv = 1.0 / (BASE ** (np.arange(0, DH, 2, dtype=np.float64) / DH))  # [32]
    pos = np.arange(N, dtype=np.float64)
    fr = pos[None, :] * inv[:, None]                                   # [32, N]
    cos32 = np.cos(fr)
    sin32 = np.sin(fr)
    cosf = np.tile(cos32, (4, 1)).astype(ml_dtypes.bfloat16)
    sinf = np.tile(sin32, (4, 1)).astype(ml_dtypes.bfloat16)
    return cosf, sinf


def kernel(x, mask, ln_w, ln_b, w_qkv, w_out, pm):
    bf = ml_dtypes.bfloat16
    f8 = ml_dtypes.float8_e4m3
    f = np.float32
    x = np.asarray(x, f)
    mask_b = np.asarray(mask).astype(bool)
    ln_w = np.asarray(ln_w, f)
    ln_b = np.asarray(ln_b, f)
    w_qkv = np.asarray(w_qkv, f)
    w_out = np.asarray(w_out, f)
    pm = np.asarray(pm, f)

    cosf, sinf = _rope_tables()
    w_eff = ln_w[:, None] * w_qkv                     # [512, 1536]
    brow = ln_b @ w_qkv                               # [1536]

    in_maps = []
    for c in range(NCORES):
        bc = c // 4
        g0, g1 = (c % 4) * 2, (c % 4) * 2 + 1
        qcols = lambda g, lo, hi: np.arange(g * 64 + lo, g * 64 + hi)
        acols = np.concatenate([qcols(g0, 0, 32), qcols(g1, 0, 32),
                                512 + qcols(g0, 0, 32), 512 + qcols(g1, 0, 32)])
        bcols = np.concatenate([qcols(g0, 32, 64), qcols(g1, 32, 64),
                                512 + qcols(g0, 32, 64), 512 + qcols(g1, 32, 64)])
        vcols = np.concatenate([1024 + qcols(g0, 0, 64), 1024 + qcols(g1, 0, 64)])
        cols = np.concatenate([acols, bcols, vcols])  # [384]
        wq = np.zeros((5 * 128, 384), f)
        wq[0:512] = w_eff[:, cols]
        wq[512] = brow[cols]
        # pm[0, g] : [NPM, 64] -> fp8 DoubleRow layout [32, h, j, t], d = p+32j
        pmk = np.zeros((32, 2, 2, NPM), f)
        for hh, g in enumerate((g0, g1)):
            kk = pm[0, g].T                           # [64, NPM]
            pmk[:, hh, 0, :] = kk[0:32]
            pmk[:, hh, 1, :] = kk[32:64]
        pmv = np.zeros((NPM, 130), f)
        pmv[:, 0:64] = pm[1, g0]
        pmv[:, 64] = 1.0
        pmv[:, 65:129] = pm[1, g1]
        pmv[:, 129] = 1.0
        mk = mask_b[bc, 0].astype(f).reshape(NBLK, 128).T  # [128, NBLK]
        in_maps.append(dict(
            x_in=x[bc].astype(bf),
            wq_in=wq.astype(bf),
            wo0_in=w_out[g0 * 64:(g0 + 1) * 64].astype(bf),
            wo1_in=w_out[g1 * 64:(g1 + 1) * 64].astype(bf),
            cos_in=cosf, sin_in=sinf,
            pmk_in=pmk.astype(f8), pmv_in=pmv.astype(bf),
            msk_in=np.ascontiguousarray(mk),
            ones_in=np.ones((1, N), bf),
            id_in=np.eye(128, dtype=bf),
        ))

    global _LAST_IN_MAPS
    _LAST_IN_MAPS = in_maps
    nc = _get_nc()
    res = run_bass_kernel_spmd(nc, in_maps, core_ids=list(range(NCORES)))
    out = np.zeros((B, N, D), f)
    for c in range(NCORES):
        out[c // 4] += res.results[c]["out_p"]
    return out

